# revision 24
# baseline (speedup 1.0000x reference)
"""Trainium2 Bass kernel for a 2-layer cosine-similarity attention GCN.

Reference math (per (b,h) slice, two chained blocks):
    xn = x / max(||x||_row, eps)
    A  = softmax((xn @ xn^T) / max(alpha, 0.01), axis=-1)
    out = relu((A @ x) @ W^T + x)

Shapes: x [4, 4, 4096, 64] fp32; W [64, 64]. B*H = 16 slices sharded as
2 slices per NeuronCore across 8 cores (fully independent, no collectives).

Kernel strategy (per core, 2 pairs x 2 blocks, all on-chip):
  - logits are cosine sims in [-1,1]*scale -> softmax without max-subtraction:
    P = exp(S*scale)/Z. E is materialized in fp8e4 (j-on-free orientation via
    the symmetry E^T == E), and U = [x|1]^T E is computed with fp8 DoubleRow
    matmuls (K=256 per instruction, M=80: 64 x-dims + ones col + 15 pad for
    the dual-fp8 LDWEIGHTS step%16 rule). Row 64 of U gives Z for free.
  - exp is split between the ACT engine (native Exp -> fp8 out) and the DVE
    (Schraudolph bit-trick: fp8e4 bits = rne(S*scale*8*log2e + 55.55) written
    as int8), so neither engine is the wall.
  - division by Z is deferred past the W matmul (per-row scale commutes with
    right-multiplication), applied after a PE transpose of [G; Z].
  - chunk epilogues are interleaved into the next chunk's main loop so the
    PE instruction stream stays dense (HAM clock-gate wants sustained busy).
  - row 1/||x|| uses a fast inverse sqrt (bit trick + 3 Newton steps) on the
    vector engine; normalized bf16 rows are produced by ACT Copy-with-scale.
"""

import numpy as np

import concourse.bacc as bacc
import concourse.tile as tile
from concourse import mybir
from concourse.bass_utils import run_bass_kernel_spmd
from concourse.masks import make_identity

F32 = mybir.dt.float32
I8 = mybir.dt.int8
I32 = mybir.dt.int32
BF16 = mybir.dt.bfloat16
FP8 = mybir.dt.float8e4
AF = mybir.ActivationFunctionType
ALU = mybir.AluOpType
DRMODE = mybir.MatmulPerfMode.DoubleRow

P = 128
D = 64
MDR = 80          # DR stationary cols: 64 x | 1 ones | 15 pad (step%16==0)
N_CORES = 8
B_EXP = 55.55     # calibrated Schraudolph offset for fp8e4 (RNE int convert)
ACT_NUM, ACT_DEN = 8, 16   # fraction of exp tiles routed to the ACT engine
INTERLEAVE = False         # run prev-chunk epilogues inside the next chunk loop


def build_nc(scales, n_rows=4096, npairs=2):
    nblocks = len(scales)
    NT = n_rows // P          # 32 row tiles
    NBP = NT // 2             # 16 row-tile pairs (DR K=256)
    CHW = min(1024, n_rows)   # j-chunk width
    NCH = n_rows // CHW
    HALF = 512                # fp32 PSUM bank width
    NH = CHW // HALF
    TPH = HALF // P           # 4 transpose pieces per half

    nc = bacc.Bacc("TRN2", target_bir_lowering=False, debug=False, num_devices=N_CORES)
    xin = nc.dram_tensor("xin", [npairs, n_rows, D], F32, kind="ExternalInput").ap()
    wts = [
        nc.dram_tensor(f"w{i}t", [D, D], F32, kind="ExternalInput").ap()
        for i in range(nblocks)
    ]
    out = nc.dram_tensor("out", [npairs, n_rows, D], F32, kind="ExternalOutput").ap()

    xin_t = xin.rearrange("p (t pp) d -> p pp t d", pp=P)  # [np, 128, NT, 64]
    out_t = out.rearrange("p (t pp) d -> p pp t d", pp=P)

    with tile.TileContext(nc) as tc:
        with (
            tc.tile_pool(name="singles", bufs=1) as singles,
            tc.tile_pool(name="stats", bufs=2) as stats,
            tc.tile_pool(name="tmp", bufs=6) as tmp,
            tc.tile_pool(name="epool", bufs=4) as epool,
            tc.tile_pool(name="fin", bufs=6) as fin,
            tc.tile_pool(name="ps_s", bufs=2, space="PSUM") as ps_s,
            tc.tile_pool(name="ps_uz", bufs=1, space="PSUM") as ps_uz,
        ):
            ident16 = singles.tile([P, P], BF16, tag="ident16")
            make_identity(nc, ident16[:])

            wb16 = []
            for i in range(nblocks):
                wtmp = singles.tile([D, D], F32, tag=f"wtmp{i}", name=f"wtmp{i}")
                nc.sync.dma_start(wtmp[:], wts[i])
                w16 = singles.tile([D, D], BF16, tag=f"w16_{i}", name=f"w16_{i}")
                nc.vector.tensor_copy(w16[:], wtmp[:])
                wb16.append(w16)

            xnt = singles.tile([P, n_rows], BF16, tag="xnt", name="xnt")
            xb = {}
            xb8 = {}
            for p in range(npairs):
                for blk in range(nblocks):
                    xb[p, blk] = singles.tile(
                        [P, NT, D], F32, tag=f"xb_{p}_{blk}", name=f"xb_{p}_{blk}"
                    )
                    xb8[p, blk] = singles.tile(
                        [P, NBP, 2, MDR], FP8, tag=f"xb8_{p}_{blk}", name=f"xb8_{p}_{blk}"
                    )
                    nc.vector.memset(xb8[p, blk][:, :, :, D : D + 1], 1.0)
                    nc.vector.memset(xb8[p, blk][:, :, :, D + 1 : MDR], 0.0)

            for p in range(npairs):
                q = NT // 4
                for k in range(4):
                    nc.sync.dma_start(
                        xb[p, 0][:, k * q : (k + 1) * q],
                        xin_t[p][:, k * q : (k + 1) * q],
                    )

            exp_cnt = [0]

            def emit_exp(dst, src, scale):
                """dst: E2 fp8 slice [128, CHW]; src: S psum [128, CHW] f32."""
                k = exp_cnt[0]
                exp_cnt[0] += 1
                on_act = (k % 4) in (0, 3)  # rotate pair<->engine per g
                if on_act:
                    nc.scalar.activation(dst, src, AF.Exp, scale=scale)
                else:
                    a_exp = 8.0 * scale / np.log(2.0)
                    nc.vector.tensor_scalar(
                        out=dst.bitcast(I8), in0=src,
                        scalar1=float(a_exp), scalar2=B_EXP,
                        op0=ALU.mult, op1=ALU.add,
                    )

            MAGIC = 0x5F3759DF

            sq_scr = singles.tile([P, D], F32, tag="sq_scr", name="sq_scr")

            def emit_square(p, blk, b, s_all):
                xsl = xb[p, blk][:, b, :]
                nc.scalar.activation(
                    sq_scr[:], xsl, AF.Square,
                    accum_out=s_all[:, b : b + 1],
                )

            def prep(p, blk, s_all=None):
                """Row norms -> 1/||x||, normalized bf16 rows -> PE transpose
                into xnt; block-0 also casts x -> fp8 DR stationary layout.
                If s_all is given, the squares were already emitted (fused into
                the previous block's epilogue)."""
                if s_all is None:
                    s_all = stats.tile([P, NT], F32, tag="s_all")
                    for b in range(NT):
                        if blk == 0:
                            nc.gpsimd.tensor_copy(
                                xb8[p, blk][:, b // 2, b % 2, 0:D],
                                xb[p, blk][:, b, :],
                            )
                        emit_square(p, blk, b, s_all)
                nc.vector.tensor_scalar_max(s_all[:], s_all[:], 1e-24)
                # rinv = s^-0.5 via fast-inverse-sqrt seed + 3 Newton steps.
                r = stats.tile([P, NT], F32, tag="rinv")
                s_i = s_all[:].bitcast(I32)
                r_i = r[:].bitcast(I32)
                nc.vector.tensor_scalar(
                    out=r_i, in0=s_i, scalar1=1, scalar2=None,
                    op0=ALU.logical_shift_right,
                )
                nc.vector.tensor_scalar(
                    out=r_i, in0=r_i, scalar1=MAGIC, scalar2=None, op0=ALU.subtract
                )
                nc.vector.tensor_scalar(
                    out=r_i, in0=r_i, scalar1=-1, scalar2=None, op0=ALU.bitwise_xor
                )
                nc.vector.tensor_scalar(
                    out=r_i, in0=r_i, scalar1=1, scalar2=None, op0=ALU.add
                )
                t1 = stats.tile([P, NT], F32, tag="nt1")
                for _ in range(3):
                    nc.vector.tensor_mul(t1[:], r[:], r[:])
                    nc.vector.tensor_mul(t1[:], t1[:], s_all[:])
                    nc.vector.tensor_scalar(
                        out=t1[:], in0=t1[:], scalar1=-0.5, scalar2=1.5,
                        op0=ALU.mult, op1=ALU.add,
                    )
                    nc.vector.tensor_mul(r[:], r[:], t1[:])
                lo = D * p
                fused = s_all is not None
                for b in range(NT):
                    xn16 = tmp.tile([P, D], BF16, tag="xn16")
                    nc.vector.tensor_scalar_mul(
                        xn16[:], xb[p, blk][:, b, :], r[:, b : b + 1]
                    )
                    pst = ps_s.tile([P, P], BF16, tag="S")
                    nc.tensor.transpose(pst[lo : lo + D, :], xn16[:], ident16[:])
                    if fused and b % 2 == 0:
                        nc.scalar.activation(
                            xnt[lo : lo + D, b * P : (b + 1) * P],
                            pst[lo : lo + D, :], AF.Copy,
                        )
                    else:
                        nc.vector.tensor_copy(
                            xnt[lo : lo + D, b * P : (b + 1) * P],
                            pst[lo : lo + D, :],
                        )

            def make_closures(blk, a, uz, last, s_next=None):
                """Epilogue for chunk (blk, a): per (pair, half) prologue
                [UTf copy, G=W@U into the same psum, GZ pack] then 4 transpose
                pieces [T, 1/Z, scale, +x, relu, casts]."""
                closures = []

                def utf_phase(p, h, utfref):
                    def run():
                        u = uz[p, h]
                        utf = fin.tile([D, HALF], BF16, tag="UTf")
                        nc.scalar.activation(utf[:], u[0:D, :], AF.Copy)
                        utfref[0] = utf
                    return run

                def g_phase(p, h, utfref, gzref):
                    def run():
                        u = uz[p, h]
                        nc.tensor.matmul(
                            u[0:D, :], lhsT=wb16[blk][:], rhs=utfref[0][:],
                            start=True, stop=True,
                        )
                        gz = fin.tile([D + 1, HALF], BF16, tag="GZ")
                        nc.scalar.activation(gz[:], u[0 : D + 1, :], AF.Copy)
                        gzref[0] = gz
                    return run

                def tpiece(p, gzref, gi, t):
                    def run():
                        gz = gzref[0]
                        T = ps_s.tile([P, D + 1], BF16, tag="S")
                        nc.tensor.transpose(
                            T[:], gz[:, t * P : (t + 1) * P],
                            ident16[0 : D + 1, 0 : D + 1],
                        )
                        rz = tmp.tile([P, 1], F32, tag="rz")
                        nc.vector.reciprocal(rz[:], T[:, D : D + 1])
                        tmpo = tmp.tile([P, D], F32, tag="tmpo")
                        nc.vector.scalar_tensor_tensor(
                            out=tmpo[:], in0=T[:, 0:D], scalar=rz[:],
                            in1=xb[p, blk][:, gi, :],
                            op0=ALU.mult, op1=ALU.add,
                        )
                        if last:
                            oo = tmp.tile([P, D], F32, tag="oo")
                            nc.vector.tensor_scalar_max(oo[:], tmpo[:], 0.0)
                            nc.sync.dma_start(out_t[p][:, gi, :], oo[:])
                        else:
                            dst = xb[p, blk + 1][:, gi, :]
                            nc.vector.tensor_scalar_max(dst, tmpo[:], 0.0)
                            nc.gpsimd.tensor_copy(
                                xb8[p, blk + 1][:, gi // 2, gi % 2, 0:D], dst
                            )
                            if s_next is not None:
                                emit_square(p, blk + 1, gi, s_next[p])
                    return run

                mids = []
                tails = []
                for p in range(npairs):
                    for h in range(NH):
                        utfref = [None]
                        gzref = [None]
                        closures.append(utf_phase(p, h, utfref))
                        mids.append(g_phase(p, h, utfref, gzref))
                        for t in range(TPH):
                            gi = a * (CHW // P) + h * TPH + t
                            tails.append(tpiece(p, gzref, gi, t))
                return closures + mids + tails

            def emit_u(blk, uz, bp, E2, start, stop):
                for p in range(npairs):
                    for h in range(NH):
                        nc.tensor.matmul(
                            uz[p, h][:],
                            lhsT=xb8[p, blk][:, bp],
                            rhs=E2[p][:, :, h * HALF : (h + 1) * HALF],
                            start=start, stop=stop,
                            perf_mode=DRMODE,
                        )

            ULAG = 2
            pending = []
            s_next = None
            for blk in range(nblocks):
                scale = scales[blk]
                for p in range(npairs):
                    prep(p, blk, s_all=None if s_next is None else s_next[p])
                s_next = None
                if blk != nblocks - 1:
                    s_next = {
                        p: stats.tile([P, NT], F32, tag="s_all",
                                      name=f"s_all_n{p}_{blk}")
                        for p in range(npairs)
                    }
                for a in range(NCH):
                    uz = None
                    e2q = []   # (bp, E2 dict) awaiting their U matmuls
                    for bp in range(NBP):
                        E2 = {
                            p: epool.tile([P, 2, CHW], FP8, tag=f"E2_{p}",
                                          name=f"E2_{blk}_{a}_{bp}_{p}")
                            for p in range(npairs)
                        }
                        for g in range(2):
                            b = 2 * bp + g
                            for p in range(npairs):
                                lo = D * p
                                S = ps_s.tile([P, CHW], F32, tag="S")
                                for h in range(NH):
                                    nc.tensor.matmul(
                                        S[:, h * HALF : (h + 1) * HALF],
                                        lhsT=xnt[lo : lo + D, b * P : (b + 1) * P],
                                        rhs=xnt[
                                            lo : lo + D,
                                            a * CHW + h * HALF : a * CHW + (h + 1) * HALF,
                                        ],
                                        start=True, stop=True,
                                    )
                                emit_exp(E2[p][:, g, :], S[:], scale)
                        e2q.append((bp, E2))
                        if INTERLEAVE:
                            if bp == 0:
                                # all prologues of the previous chunk, before
                                # the first U here re-allocates the UZ slots
                                n_pro = min(len(pending), npairs * NH)
                                for _ in range(n_pro):
                                    pending.pop(0)()
                            else:
                                budget = 2 if bp % 2 else 1
                                for _ in range(min(len(pending), budget)):
                                    pending.pop(0)()
                        if bp >= ULAG:
                            if uz is None:
                                uz = {
                                    (p, h): ps_uz.tile(
                                        [MDR, HALF], F32, tag=f"UZ_{p}_{h}",
                                        name=f"UZ_{blk}_{a}_{p}_{h}",
                                    )
                                    for p in range(npairs)
                                    for h in range(NH)
                                }
                            qbp, qE2 = e2q.pop(0)
                            emit_u(blk, uz, qbp, qE2,
                                   start=(qbp == 0), stop=False)
                    # drain leftovers of previous chunk, then remaining U
                    while pending:
                        pending.pop(0)()
                    while e2q:
                        qbp, qE2 = e2q.pop(0)
                        emit_u(blk, uz, qbp, qE2,
                               start=(qbp == 0), stop=(qbp == NBP - 1))
                    pending = make_closures(
                        blk, a, uz, last=(blk == nblocks - 1),
                        s_next=s_next,
                    )
                    if not INTERLEAVE:
                        while pending:
                            pending.pop(0)()
                # block boundary: next prep reads xb[blk+1] -> drain epilogues
                if blk != nblocks - 1:
                    while pending:
                        pending.pop(0)()
            while pending:
                pending.pop(0)()

    nc.compile()
    return nc


_CACHE = {}


def _get_nc(scales, n_rows, npairs):
    key = (tuple(scales), n_rows, npairs)
    if key not in _CACHE:
        _CACHE[key] = build_nc(list(scales), n_rows=n_rows, npairs=npairs)
    return _CACHE[key]


def kernel(x, W1, W2, alpha1, alpha2):
    x = np.asarray(x, dtype=np.float32)
    B, H, N, d = x.shape
    assert d == D and (B * H) % N_CORES == 0
    npairs = (B * H) // N_CORES
    s1 = 1.0 / max(float(alpha1), 0.01)
    s2 = 1.0 / max(float(alpha2), 0.01)
    nc = _get_nc((s1, s2), N, npairs)

    xf = np.ascontiguousarray(x.reshape(B * H, N, d))
    w0 = np.ascontiguousarray(np.asarray(W1, dtype=np.float32).T)
    w1 = np.ascontiguousarray(np.asarray(W2, dtype=np.float32).T)
    in_maps = [
        {"xin": xf[npairs * c : npairs * (c + 1)], "w0t": w0, "w1t": w1}
        for c in range(N_CORES)
    ]
    res = run_bass_kernel_spmd(nc, in_maps, core_ids=list(range(N_CORES)))
    outs = np.stack([r["out"] for r in res.results])
    return outs.reshape(B, H, N, d).astype(np.float32)
